# revision 4
# baseline (speedup 1.0000x reference)
"""Self-cdist kernel for Trainium2 (8 NeuronCores, Bass/Tile).

Computes the full [2048, 2048] pairwise Euclidean distance matrix of
x [2048, 64] f32, sharded row-wise across 8 cores (256 query rows per
core, every core holds all of x).

Math per core: d(i,j) = sqrt(s_i + s_j - 2 * x_i . x_j) with
  - s_j broadcast folded into the matmul via an augmented contraction
    row (K = 65: rows 0..63 = x^T, row 64 = s_row / ones)
  - s_i added as the per-partition bias of the ScalarE Sqrt activation
  - the diagonal (which is ~0 +/- fp rounding and may go negative)
    zeroed exactly with a gpsimd affine_select.

SPMD trick: every core runs the identical program; core c receives
x rolled by -256*c rows (transposed to [64, 2048]), so its queries are
always local rows 0..255 and the diagonal always sits at local (r, r).
The host un-rolls the columns when assembling the full output.
"""

import sys

if "/opt/trn_rl_repo" not in sys.path:
    sys.path.insert(0, "/opt/trn_rl_repo")

import numpy as np

N, D = 2048, 64
NCORES = 8
Q = N // NCORES          # 256 query rows per core
P = 128                  # SBUF partitions per row-chunk
NCHUNK = Q // P          # 2 row chunks per core
CT = 512                 # output column tile (one PSUM bank of fp32)
NCT = N // CT            # 4 column tiles

# Matmul input precision: float32r is ~4x faster on the PE but reduced
# precision; plain float32 is exact. Switched after HW measurement.
USE_F32R = True

_cached_nc = None


def _build():
    import concourse.bacc as bacc
    import concourse.tile as tile
    from concourse import mybir

    f32 = mybir.dt.float32
    dt_mm = mybir.dt.float32r if USE_F32R else f32
    AF = mybir.ActivationFunctionType

    nc = bacc.Bacc("TRN2", target_bir_lowering=False, debug=False,
                   num_devices=NCORES)
    xt = nc.dram_tensor("xt", [D, N], f32, kind="ExternalInput").ap()
    out = nc.dram_tensor("out", [Q, N], f32, kind="ExternalOutput").ap()

    with tile.TileContext(nc) as tc:
        with (
            tc.tile_pool(name="const", bufs=1) as cpool,
            tc.tile_pool(name="outp", bufs=2) as opool,
            tc.tile_pool(name="mm_ps", bufs=4, space="PSUM") as mm_pool,
            tc.tile_pool(name="s_ps", bufs=2, space="PSUM") as s_pool,
            tc.tile_pool(name="b_ps", bufs=2, space="PSUM") as b_pool,
        ):
            # Tiny dummy activation up front so walrus' ACT table load
            # (sqrt set, ~2.7us) overlaps the input DMA instead of
            # stalling the first real sqrt.
            dummy = cpool.tile([P, 1], f32)
            nc.vector.memset(dummy[:], 1.0)
            nc.scalar.activation(dummy[:], dummy[:], AF.Sqrt)

            # X_aug rows 0..63 = x^T (rolled), row 64 = s_row (filled below).
            # Norm helpers (X2, ones) stay plain f32: their matmuls are
            # tiny/odd-shaped and fp32 ISA rules are laxer; only the 8 big
            # output matmuls use dt_mm.
            X_aug = cpool.tile([D + 1, N], dt_mm)
            X2 = cpool.tile([D, N], f32)
            ones_col = cpool.tile([D, 2], f32)
            nc.gpsimd.memset(ones_col[:], 1.0)
            if USE_F32R:
                X_stage = cpool.tile([D, N], f32)

            # Load + square + row-norms, pipelined by column tile.
            for t in range(NCT):
                cs = slice(t * CT, (t + 1) * CT)
                if USE_F32R:
                    # f32r operands must be produced (rounded) by an
                    # engine; DMA lands f32, DVE copy rounds to f32r.
                    nc.sync.dma_start(X_stage[:, cs], xt[:, cs])
                    nc.vector.tensor_copy(X_aug[0:D, cs], X_stage[:, cs])
                    nc.vector.tensor_mul(X2[:, cs], X_stage[:, cs],
                                         X_stage[:, cs])
                else:
                    nc.sync.dma_start(X_aug[0:D, cs], xt[:, cs])
                    nc.vector.tensor_mul(X2[:, cs], X_aug[0:D, cs],
                                         X_aug[0:D, cs])
                s_ps = s_pool.tile([1, CT], f32)
                nc.tensor.matmul(s_ps[:], ones_col[:, 0:1], X2[:, cs],
                                 start=True, stop=True)
                nc.vector.tensor_copy(X_aug[D:D + 1, cs], s_ps[:])

            # lhsT: rows 0..63 = -2 * queries^T, row 64 = ones (-> + s_j)
            qs_aug = cpool.tile([D + 1, Q], dt_mm)
            nc.vector.tensor_scalar_mul(qs_aug[0:D, :], X_aug[0:D, 0:Q], -2.0)
            ones_row = cpool.tile([1, Q], f32)
            nc.vector.memset(ones_row[:], 1.0)
            nc.vector.tensor_copy(qs_aug[D:D + 1, :], ones_row[:])

            # Per-chunk bias column: s_i for the 128 queries of the chunk.
            biases = []
            for c in range(NCHUNK):
                qs_ = slice(c * P, (c + 1) * P)
                b_ps = b_pool.tile([P, 2], f32)
                nc.tensor.matmul(b_ps[:], X2[:, qs_], ones_col[:, 0:2],
                                 start=True, stop=True)
                bias_sb = cpool.tile([P, 1], f32, tag="bias", name=f"bias{c}")
                # +1e-3 keeps the (~0 +/- rounding) diagonal non-negative
                # for sqrt; it is zeroed exactly afterwards, and off-diag
                # d^2 >= ~46 so the distortion is < 1.1e-5 relative.
                nc.vector.tensor_scalar_add(bias_sb[:], b_ps[:, 0:1], 1e-3)
                biases.append(bias_sb)

            # Main: one K=65 matmul per [128, 512] output tile, then
            # sqrt(psum + s_i) on ScalarE straight out of PSUM.
            for c in range(NCHUNK):
                qs_ = slice(c * P, (c + 1) * P)
                out_sb = opool.tile([P, N], f32)
                for t in range(NCT):
                    cs = slice(t * CT, (t + 1) * CT)
                    mm_ps = mm_pool.tile([P, CT], f32)
                    nc.tensor.matmul(mm_ps[:], qs_aug[:, qs_], X_aug[:, cs],
                                     start=True, stop=True)
                    nc.scalar.activation(out_sb[:, cs], mm_ps[:], AF.Sqrt,
                                         bias=biases[c][:], scale=1.0)
                # Exact-zero the diagonal stripe (local (r, r) lands in
                # columns [c*128, c*128+128) for this chunk).
                ds_ = slice(c * P, (c + 1) * P)
                nc.gpsimd.affine_select(
                    out=out_sb[:, ds_], in_=out_sb[:, ds_],
                    compare_op=mybir.AluOpType.not_equal, fill=0.0,
                    base=0, pattern=[[-1, P]], channel_multiplier=1,
                )
                nc.sync.dma_start(out[c * P:(c + 1) * P, :], out_sb[:])

    nc.compile()
    return nc


def _get_nc():
    global _cached_nc
    if _cached_nc is None:
        _cached_nc = _build()
    return _cached_nc


def kernel(x: np.ndarray) -> np.ndarray:
    from concourse import bass_utils

    x = np.ascontiguousarray(np.asarray(x, dtype=np.float32))
    assert x.shape == (N, D), x.shape

    nc = _get_nc()
    in_maps = [
        {"xt": np.ascontiguousarray(np.roll(x, -Q * c, axis=0).T)}
        for c in range(NCORES)
    ]
    res = bass_utils.run_bass_kernel_spmd(nc, in_maps,
                                          core_ids=list(range(NCORES)))
    full = np.empty((N, N), dtype=np.float32)
    for c in range(NCORES):
        # local col j of core c is global row (j + Q*c) % N -> roll back
        full[Q * c:Q * (c + 1), :] = np.roll(res.results[c]["out"], Q * c,
                                             axis=1)
    return full
